# revision 21
# baseline (speedup 1.0000x reference)
"""Supervised-contrastive loss on 8 TRN2 NeuronCores — v8.

Math (identical to the reference):
    s_ij  = cosine similarity of feature rows i, j
    E_ij  = exp(s_ij / tau)
    neg_i = sum_j E_ij * (1 - mask_ij)          (mask = same-class, incl diag)
    loss  = sum_{i, same-class j != i} [ln(E_ij + neg_i) - s_ij/tau] / p_i
            ---------------------------------------------------------------
                                     sum_i p_i

Key ideas:
  * Rows are SORTED BY CLASS on the host, so every same-class pair (i, j)
    satisfies |i - j| < 128.  All mask work and the ln() pass then touch
    only a W=256-column diagonal band instead of the full 4096 columns.
  * The GEMM runs in fp8 (e4m3, x64 pre-scale) with DoubleRow perf mode:
    256-deep contraction per matmul, half the matmul count of bf16.
  * Each core receives a column-ROTATED copy of fnT8 (own block at local
    columns [512, 1024)), which makes the program core-independent; the
    band wrap-around columns carry zero masks, so they only contribute
    ln(neg) terms that the host subtracts in closed form.
  * exp and ln share one ACT table set (natural_log_exp_and_others), so
    the per-row-tile ln can interleave with exp at zero switch cost.
  * DMA pieces are contiguous on BOTH the DRAM and SBUF side (2-8 KB
    descriptors) and ordered by first use: own block, prev block, rest of
    half 0, half 1.  DMA throughput is descriptor-bound, so descriptor
    size is everything.
  * ~2us of dummy matmuls on a zeroed tile warm the PE clock (HAM) while
    the first DMA pieces are still in flight.
  * The mask covers the diagonal; the host subtracts its ln term in
    closed form (E_ii = exp(|fn8_i|^2/(tau*S^2)) is host-computable).

Device outputs per row: lnsum_i (band ln-sum incl diag) and neg_i.
Host postprocess (O(N*D)):
    A_i  = lnsum_i - (W - p_i)*ln(neg_i) - ln(E_ii + neg_i)
    B_i  = (fnq_i . g(class_i) - |fnq_i|^2) / tau
    loss = sum((A - B)/p) / sum(p)
"""

import numpy as np
import ml_dtypes

TAU = 0.1
N, D = 4096, 512
NCORES = 8
ROWS = N // NCORES          # 512 rows per core
IT = ROWS // 128            # 4 partition tiles per core
W = 256                     # band width (covers class sizes <= 65)
PAD = 64                    # band left-overhang
S8 = 64.0                   # fp8 pre-scale

_CACHE = {}

# SBUF piece layout: [128, 2(kp), 2(i), cols]; local col ranges per piece.
PIECES = [("pA", 512, 1024), ("pB", 0, 512), ("pC", 1024, 2048),
          ("pD", 2048, 4096)]


def _build_nc():
    import concourse.tile as tile
    import concourse.mybir as mybir
    from concourse import bacc

    dt = mybir.dt
    AF = mybir.ActivationFunctionType
    ALU = mybir.AluOpType
    AX = mybir.AxisListType
    PM = mybir.MatmulPerfMode

    # Force Exp AND Ln to resolve to the one table set that holds both, so
    # a single ACT_TABLE_LOAD serves the whole kernel.  Entries keep their
    # original indices (ids index act_info.json) — we only blank the
    # Exp/Ln membership of the competing sets during this build.
    orig_get = bacc.get_activation_tables

    def patched(arch):
        out = {}
        for name, fns in orig_get(arch).items():
            if name != "natural_log_exp_and_others" and (
                AF.Exp in fns or AF.Ln in fns
            ):
                fns = {f for f in fns if f not in (AF.Exp, AF.Ln)}
            out[name] = fns
        return out

    bacc.get_activation_tables = patched
    try:
        nc = bacc.Bacc(None)
        dram_p = {
            name: nc.declare_dram_parameter(
                name, [128, 2, 2, c1 - c0], dt.float8e4, isOutput=False)
            for name, c0, c1 in PIECES
        }
        m1 = nc.declare_dram_parameter("m1", [128, IT * W], dt.float8e4, isOutput=False)
        out2 = nc.declare_dram_parameter("out2", [128, 2 * IT], dt.float32, isOutput=True)

        with tile.TileContext(nc) as tc:
            with (
                tc.tile_pool(name="persist", bufs=1) as persist,
                tc.tile_pool(name="psum", bufs=2, space="PSUM") as psum,
                tc.tile_pool(name="acc", bufs=2) as accp,
                tc.tile_pool(name="band", bufs=2) as bandp,
                tc.tile_pool(name="outp", bufs=1) as outp,
            ):
                # ---- persistent SBUF ----
                P = {
                    name: persist.tile([128, 2, 2, c1 - c0], dt.float8e4,
                                       name=name, tag=name)
                    for name, c0, c1 in PIECES
                }
                M1s = persist.tile([128, IT * W], dt.float8e4, tag="m1")
                rsE2 = [accp.tile([128, 5], dt.float32, name=f"rse2_{it}",
                                  tag=f"rse2_{it}") for it in range(IT)]
                E = [persist.tile([128, N], dt.bfloat16, name=f"e{it}",
                                  tag=f"e{it}") for it in range(IT)]
                out_sb = outp.tile([128, 2 * IT], dt.float32, tag="out2")

                # ---- PE warm-up: zeroed dummies keep the HAM busy while
                # the first DMA pieces land; sized so they drain right as
                # the first real matmul's data arrives.
                wz = persist.tile([128, 2, 640], dt.float8e4, tag="wz")
                zb = persist.tile([128, 1], dt.float32, tag="zb")
                with tc.high_priority():
                    nc.vector.memset(wz[:], 0)
                    nc.vector.memset(zb[:], 0)
                for it in range(IT):
                    nc.vector.memset(rsE2[it][:], 0)
                wps = psum.tile([128, 2048], dt.float32, tag="S")
                for _ in range(6):
                    nc.tensor.matmul(
                        wps[:, 0:512], wz[:, :, 0:128], wz[:, :, 128:640],
                        start=True, stop=True, perf_mode=PM.DoubleRow,
                    )

                # ---- DMA, in first-use order, all on the sync queue ----
                with tc.high_priority():
                    nc.scalar.dma_start(P["pA"][:], dram_p["pA"][:])
                    nc.sync.dma_start(P["pB"][:], dram_p["pB"][:])
                nc.sync.dma_start(P["pC"][:], dram_p["pC"][:])
                nc.sync.dma_start(P["pD"][:], dram_p["pD"][:])
                nc.sync.dma_start(M1s[:], m1[:])

                def rhs(kp, c0, c1):
                    """[128, 2, c1-c0] fp8 view of local cols [c0, c1)."""
                    for name, p0, p1 in PIECES:
                        if p0 <= c0 and c1 <= p1:
                            return P[name][:, kp, :, c0 - p0:c1 - p0]
                    raise AssertionError((c0, c1))

                lhsT = [[rhs(kp, 512 + it * 128, 640 + it * 128)
                         for it in range(IT)] for kp in range(2)]

                EXP_SCALE = 1.0 / (TAU * S8 * S8)

                def gemm_exp(it, h, split_exp=False):
                    Sh = psum.tile([128, 2048], dt.float32, tag="S")
                    base = h * 2048
                    if h == 0:
                        # chunk-outer order: cols [0,1024) (pieces A+B) finish
                        # after 4 matmuls, so their exp doesn't wait on pC.
                        # Only the first two tiles race the pC DMA; later
                        # tiles use one whole-width exp (fewer accum reads).
                        split_h0 = it < 2
                        if split_h0:
                            for q in (1, 0, 2, 3):
                                for kp in range(2):
                                    c0 = q * 512
                                    nc.tensor.matmul(
                                        Sh[:, q * 512:(q + 1) * 512],
                                        lhsT[kp][it],
                                        rhs(kp, c0, c0 + 512),
                                        start=(kp == 0),
                                        stop=(kp == 1),
                                        perf_mode=PM.DoubleRow,
                                    )
                                if q == 0:
                                    nc.scalar.activation(
                                        E[it][:, 0:1024], Sh[:, 0:1024], AF.Exp,
                                        bias=zb[:, 0:1], scale=EXP_SCALE,
                                        accum_out=rsE2[it][:, 0:1])
                        else:
                            # kp-outer: one weight load per kp (8 matmuls)
                            for kp in range(2):
                                for q in (1, 0, 2, 3):
                                    c0 = q * 512
                                    nc.tensor.matmul(
                                        Sh[:, q * 512:(q + 1) * 512],
                                        lhsT[kp][it],
                                        rhs(kp, c0, c0 + 512),
                                        start=(kp == 0),
                                        stop=(kp == 1),
                                        perf_mode=PM.DoubleRow,
                                    )
                        if split_h0:
                            nc.scalar.activation(
                                E[it][:, 1024:2048], Sh[:, 1024:2048], AF.Exp,
                                bias=zb[:, 0:1], scale=EXP_SCALE,
                                accum_out=rsE2[it][:, 1:2])
                        else:
                            nc.scalar.activation(
                                E[it][:, 0:2048], Sh[:], AF.Exp,
                                bias=zb[:, 0:1], scale=EXP_SCALE,
                                accum_out=rsE2[it][:, 1:2])
                        return 3
                    for kp in range(2):
                        for q in (0, 1, 2, 3):
                            c0 = base + q * 512
                            nc.tensor.matmul(
                                Sh[:, q * 512:(q + 1) * 512],
                                lhsT[kp][it],
                                rhs(kp, c0, c0 + 512),
                                start=(kp == 0),
                                stop=(kp == 1),
                                perf_mode=PM.DoubleRow,
                            )
                    if not split_exp:
                        nc.scalar.activation(
                            E[it][:, base:base + 2048], Sh[:], AF.Exp,
                            bias=zb[:, 0:1], scale=EXP_SCALE, accum_out=rsE2[it][:, 2:3])
                        return 3
                    # last tile: split the final exp so the end-of-kernel
                    # dependency chain is one 1024-wide exp shorter.
                    nc.scalar.activation(
                        E[it][:, base:base + 1024], Sh[:, 0:1024], AF.Exp,
                        bias=zb[:, 0:1], scale=EXP_SCALE, accum_out=rsE2[it][:, 2:3])
                    nc.scalar.activation(
                        E[it][:, base + 1024:base + 2048], Sh[:, 1024:2048],
                        AF.Exp, bias=zb[:, 0:1], scale=EXP_SCALE, accum_out=rsE2[it][:, 3:4])
                    return 4

                # ---- pass 1: local half 0 (contains the whole band) ----
                # band = local cols [448 + it*128, +W); masked products run
                # as soon as this tile's half-0 exp lands.
                band_st = []

                def band_mul(it):
                    # negEM = -(E * mask); its row sum lands in rsE2 col 4 so
                    # a single row reduce of rsE2 yields neg directly.
                    Eb = E[it][:, 448 + it * 128: 448 + it * 128 + W]
                    EM1 = bandp.tile([128, W], dt.bfloat16, tag=f"em1_{it}")
                    nc.vector.scalar_tensor_tensor(
                        EM1[:], Eb, -1.0, M1s[:, it * W:(it + 1) * W],
                        ALU.mult, ALU.mult, accum_out=rsE2[it][:, 4:5],
                    )
                    band_st.append(EM1)

                # ---- pass 2 defs (half 1; neg + band ln trail) ----
                negs = {}

                def neg_calc(it, ncols):
                    neg_t = accp.tile([128, 1], dt.float32, tag=f"neg_{it}")
                    nc.vector.tensor_reduce(
                        neg_t[:], rsE2[it][:], AX.X, ALU.add)
                    nc.vector.tensor_copy(out_sb[:, IT + it:IT + it + 1], neg_t[:])
                    negs[it] = neg_t

                def band_ln(it, last=False):
                    if last:
                        Lb = bandp.tile([128, W], dt.bfloat16, tag=f"lb_{it}")
                        nc.scalar.activation(
                            Lb[:], band_st[it][:], AF.Ln, scale=-1.0,
                            bias=negs[it][:, 0:1],
                            accum_out=out_sb[:, it:it + 1],
                        )
                    else:
                        Lb = bandp.tile([128, W], dt.float32, tag=f"lb_{it}")
                        nc.scalar.activation(
                            Lb[:], band_st[it][:], AF.Ln, scale=-1.0,
                            bias=negs[it][:, 0:1],
                        )
                        nc.vector.tensor_reduce(
                            out_sb[:, it:it + 1], Lb[:], AX.X, ALU.add)

                for it in range(IT):
                    gemm_exp(it, 0)
                    band_mul(it)
                for it in range(IT):
                    ncols = gemm_exp(it, 1, split_exp=(it == IT - 1))
                    neg_calc(it, ncols)
                    if it >= 1:
                        band_ln(it - 1)
                band_ln(IT - 1, last=True)
                nc.sync.dma_start(out2[:], out_sb[:])

        nc.finalize()
    finally:
        bacc.get_activation_tables = orig_get
    return nc


def _get_nc():
    if "nc" not in _CACHE:
        _CACHE["nc"] = _build_nc()
    return _CACHE["nc"]


def _host_prep(features, targets):
    f8t = ml_dtypes.float8_e4m3
    f = np.asarray(features, np.float32)
    t = np.asarray(targets).astype(np.int64)

    perm = np.argsort(t, kind="stable")
    fs, ts = f[perm], t[perm]
    rnorm = 1.0 / np.sqrt((fs.astype(np.float64) ** 2).sum(1))
    fn = (fs * rnorm[:, None].astype(np.float32)).astype(np.float32)
    fn8 = (fn * S8).astype(f8t)                     # [N, D] fp8 values
    fnT8 = np.ascontiguousarray(fn8.T)              # [D, N]

    in_maps = []
    for c in range(NCORES):
        roll = np.roll(fnT8, 512 - c * 512, axis=1)     # local col l = global (c*512-512+l) % N
        a = roll.reshape(2, 2, 128, N)                  # [kp, i, p, l]
        im = {}
        for name, c0, c1 in PIECES:
            im[name] = np.ascontiguousarray(
                a[:, :, :, c0:c1].transpose(2, 0, 1, 3))  # [p, kp, i, cols]
        # band masks, local band cols of row tile it: global (R0 - PAD + j) % N
        it_i = np.arange(IT)
        R0 = c * 512 + it_i * 128
        rows = R0[:, None] + np.arange(128)[None, :]            # [IT, p]
        g = (R0[:, None] - PAD + np.arange(W)[None, :]) % N     # [IT, j]
        m1 = (ts[rows][:, :, None] == ts[g][:, None, :])        # [IT, p, j]
        im["m1"] = np.ascontiguousarray(
            m1.transpose(1, 0, 2).reshape(128, IT * W).astype(f8t))
        in_maps.append(im)
    return (fn8, ts), in_maps


def _band_covered(ts):
    """Every same-class pair must fall inside the band (guaranteed for any
    remotely Poisson-like class distribution; checked for safety)."""
    cls, counts = np.unique(ts, return_counts=True)
    starts = np.zeros(len(cls) + 1, np.int64)
    starts[1:] = np.cumsum(counts)
    idx = np.searchsorted(cls, ts)
    row_lo, row_hi = starts[idx], starts[idx] + counts[idx]
    R0 = (np.arange(N) // 128) * 128
    return bool(((row_lo >= R0 - PAD) & (row_hi <= R0 - PAD + W)).all())


def _host_post(fn8, ts, lnsum_rows, neg_rows):
    cls, counts = np.unique(ts, return_counts=True)
    idx = np.searchsorted(cls, ts)
    p = counts[idx].astype(np.float64)
    Eii = np.exp((fn8.astype(np.float64) ** 2).sum(1) / (TAU * S8 * S8))
    A = (lnsum_rows - (W - p) * np.log(neg_rows)
         - np.log(Eii + neg_rows))
    fnq = fn8.astype(np.float64) / S8
    g = np.zeros((len(cls), D), np.float64)
    np.add.at(g, idx, fnq)
    B = ((fnq * g[idx]).sum(1) - (fnq ** 2).sum(1)) / TAU
    loss = ((A - B) / p).sum() / p.sum()
    return np.float32(loss)


def _rows_from_out(per_core_outs):
    lnsum = np.empty(N, np.float64)
    neg = np.empty(N, np.float64)
    for c, out in enumerate(per_core_outs):
        arr = np.asarray(out["out2"], np.float64)      # [128, 2*IT]
        lnsum[c * ROWS:(c + 1) * ROWS] = arr[:, :IT].T.reshape(ROWS)
        neg[c * ROWS:(c + 1) * ROWS] = arr[:, IT:].T.reshape(ROWS)
    return lnsum, neg


def _run(in_maps, trace=False):
    from concourse.bass_utils import run_bass_kernel_spmd
    nc = _get_nc()
    return run_bass_kernel_spmd(
        nc, in_maps, core_ids=list(range(NCORES)), trace=trace,
    )


def _numpy_fallback(features, targets):
    f = np.asarray(features, np.float64)
    t = np.asarray(targets).astype(np.int64)
    sim = f @ f.T
    nrm = np.sqrt((f ** 2).sum(1))
    nm = np.maximum(nrm[:, None] * nrm[None, :], 1e-8)
    E = np.exp(sim / nm / TAU)
    mask = (t[None, :] == t[:, None])
    np.fill_diagonal(E, 0.0)
    negv = (E * ~mask).sum(1)
    p = mask.sum(1).astype(np.float64)
    with np.errstate(divide="ignore"):
        lm = np.where(mask & (E > 0), np.log(E / (E + negv[:, None])), 0.0)
    return np.float32(-(lm / p[:, None]).sum() / p.sum())


def kernel(features, targets):
    (fn8, ts), in_maps = _host_prep(features, targets)
    if not _band_covered(ts):
        return _numpy_fallback(features, targets)
    res = _run(in_maps, trace=False)
    lnsum_rows, neg_rows = _rows_from_out(res.results)
    return _host_post(fn8, ts, lnsum_rows, neg_rows)


# revision 22
# speedup vs baseline: 1.0138x; 1.0138x over previous
"""Supervised-contrastive loss on 8 TRN2 NeuronCores — v8.

Math (identical to the reference):
    s_ij  = cosine similarity of feature rows i, j
    E_ij  = exp(s_ij / tau)
    neg_i = sum_j E_ij * (1 - mask_ij)          (mask = same-class, incl diag)
    loss  = sum_{i, same-class j != i} [ln(E_ij + neg_i) - s_ij/tau] / p_i
            ---------------------------------------------------------------
                                     sum_i p_i

Key ideas:
  * Rows are SORTED BY CLASS on the host, so every same-class pair (i, j)
    satisfies |i - j| < 128.  All mask work and the ln() pass then touch
    only a W=256-column diagonal band instead of the full 4096 columns.
  * The GEMM runs in fp8 (e4m3, x64 pre-scale) with DoubleRow perf mode:
    256-deep contraction per matmul, half the matmul count of bf16.
  * Each core receives a column-ROTATED copy of fnT8 (own block at local
    columns [512, 1024)), which makes the program core-independent; the
    band wrap-around columns carry zero masks, so they only contribute
    ln(neg) terms that the host subtracts in closed form.
  * exp and ln share one ACT table set (natural_log_exp_and_others), so
    the per-row-tile ln can interleave with exp at zero switch cost.
  * DMA pieces are contiguous on BOTH the DRAM and SBUF side (2-8 KB
    descriptors) and ordered by first use: own block, prev block, rest of
    half 0, half 1.  DMA throughput is descriptor-bound, so descriptor
    size is everything.
  * ~2us of dummy matmuls on a zeroed tile warm the PE clock (HAM) while
    the first DMA pieces are still in flight.
  * The mask covers the diagonal; the host subtracts its ln term in
    closed form (E_ii = exp(|fn8_i|^2/(tau*S^2)) is host-computable).

Device outputs per row: lnsum_i (band ln-sum incl diag) and neg_i.
Host postprocess (O(N*D)):
    A_i  = lnsum_i - (W - p_i)*ln(neg_i) - ln(E_ii + neg_i)
    B_i  = (fnq_i . g(class_i) - |fnq_i|^2) / tau
    loss = sum((A - B)/p) / sum(p)
"""

import numpy as np
import ml_dtypes

TAU = 0.1
N, D = 4096, 512
NCORES = 8
ROWS = N // NCORES          # 512 rows per core
IT = ROWS // 128            # 4 partition tiles per core
W = 256                     # band width (covers class sizes <= 65)
PAD = 64                    # band left-overhang
S8 = 64.0                   # fp8 pre-scale

_CACHE = {}

# SBUF piece layout: [128, 2(kp), 2(i), cols]; local col ranges per piece.
PIECES = [("pA", 512, 1024), ("pB", 0, 512), ("pC", 1024, 2048),
          ("pD", 2048, 4096)]


def _build_nc():
    import concourse.tile as tile
    import concourse.mybir as mybir
    from concourse import bacc

    dt = mybir.dt
    AF = mybir.ActivationFunctionType
    ALU = mybir.AluOpType
    AX = mybir.AxisListType
    PM = mybir.MatmulPerfMode

    # Force Exp AND Ln to resolve to the one table set that holds both, so
    # a single ACT_TABLE_LOAD serves the whole kernel.  Entries keep their
    # original indices (ids index act_info.json) — we only blank the
    # Exp/Ln membership of the competing sets during this build.
    orig_get = bacc.get_activation_tables

    def patched(arch):
        out = {}
        for name, fns in orig_get(arch).items():
            if name != "natural_log_exp_and_others" and (
                AF.Exp in fns or AF.Ln in fns
            ):
                fns = {f for f in fns if f not in (AF.Exp, AF.Ln)}
            out[name] = fns
        return out

    bacc.get_activation_tables = patched
    try:
        nc = bacc.Bacc(None)
        dram_p = {
            name: nc.declare_dram_parameter(
                name, [128, 2, 2, c1 - c0], dt.float8e4, isOutput=False)
            for name, c0, c1 in PIECES
        }
        m1 = nc.declare_dram_parameter("m1", [128, IT * W], dt.float8e4, isOutput=False)
        out2 = nc.declare_dram_parameter("out2", [128, 2 * IT], dt.float32, isOutput=True)

        with tile.TileContext(nc) as tc:
            with (
                tc.tile_pool(name="persist", bufs=1) as persist,
                tc.tile_pool(name="psum", bufs=2, space="PSUM") as psum,
                tc.tile_pool(name="acc", bufs=2) as accp,
                tc.tile_pool(name="band", bufs=2) as bandp,
                tc.tile_pool(name="outp", bufs=1) as outp,
            ):
                # ---- persistent SBUF ----
                P = {
                    name: persist.tile([128, 2, 2, c1 - c0], dt.float8e4,
                                       name=name, tag=name)
                    for name, c0, c1 in PIECES
                }
                M1s = persist.tile([128, IT * W], dt.float8e4, tag="m1")
                rsE2 = [accp.tile([128, 5], dt.float32, name=f"rse2_{it}",
                                  tag=f"rse2_{it}") for it in range(IT)]
                E = [persist.tile([128, N], dt.bfloat16, name=f"e{it}",
                                  tag=f"e{it}") for it in range(IT)]
                out_sb = outp.tile([128, 2 * IT], dt.float32, tag="out2")

                # ---- PE warm-up: zeroed dummies keep the HAM busy while
                # the first DMA pieces land; sized so they drain right as
                # the first real matmul's data arrives.
                wz = persist.tile([128, 2, 640], dt.float8e4, tag="wz")
                zb = persist.tile([128, 1], dt.float32, tag="zb")
                with tc.high_priority():
                    nc.vector.memset(wz[:], 0)
                    nc.vector.memset(zb[:], 0)
                for it in range(IT):
                    nc.vector.memset(rsE2[it][:], 0)
                wps = psum.tile([128, 2048], dt.float32, tag="S")
                for _ in range(6):
                    nc.tensor.matmul(
                        wps[:, 0:512], wz[:, :, 0:128], wz[:, :, 128:640],
                        start=True, stop=True, perf_mode=PM.DoubleRow,
                    )

                # ---- DMA, in first-use order, all on the sync queue ----
                with tc.high_priority():
                    nc.sync.dma_start(P["pA"][:], dram_p["pA"][:])
                    nc.sync.dma_start(P["pB"][:], dram_p["pB"][:])
                nc.sync.dma_start(P["pC"][:], dram_p["pC"][:])
                nc.sync.dma_start(P["pD"][:], dram_p["pD"][:])
                nc.sync.dma_start(M1s[:], m1[:])

                def rhs(kp, c0, c1):
                    """[128, 2, c1-c0] fp8 view of local cols [c0, c1)."""
                    for name, p0, p1 in PIECES:
                        if p0 <= c0 and c1 <= p1:
                            return P[name][:, kp, :, c0 - p0:c1 - p0]
                    raise AssertionError((c0, c1))

                lhsT = [[rhs(kp, 512 + it * 128, 640 + it * 128)
                         for it in range(IT)] for kp in range(2)]

                EXP_SCALE = 1.0 / (TAU * S8 * S8)

                def gemm_exp(it, h, split_exp=False):
                    Sh = psum.tile([128, 2048], dt.float32, tag="S")
                    base = h * 2048
                    if h == 0:
                        # chunk-outer order: cols [0,1024) (pieces A+B) finish
                        # after 4 matmuls, so their exp doesn't wait on pC.
                        # Only the first two tiles race the pC DMA; later
                        # tiles use one whole-width exp (fewer accum reads).
                        split_h0 = it < 2
                        if split_h0:
                            for q in (1, 0, 2, 3):
                                for kp in range(2):
                                    c0 = q * 512
                                    nc.tensor.matmul(
                                        Sh[:, q * 512:(q + 1) * 512],
                                        lhsT[kp][it],
                                        rhs(kp, c0, c0 + 512),
                                        start=(kp == 0),
                                        stop=(kp == 1),
                                        perf_mode=PM.DoubleRow,
                                    )
                                if q == 0:
                                    nc.scalar.activation(
                                        E[it][:, 0:1024], Sh[:, 0:1024], AF.Exp,
                                        bias=zb[:, 0:1], scale=EXP_SCALE,
                                        accum_out=rsE2[it][:, 0:1])
                        else:
                            # kp-outer: one weight load per kp (8 matmuls)
                            for kp in range(2):
                                for q in (1, 0, 2, 3):
                                    c0 = q * 512
                                    nc.tensor.matmul(
                                        Sh[:, q * 512:(q + 1) * 512],
                                        lhsT[kp][it],
                                        rhs(kp, c0, c0 + 512),
                                        start=(kp == 0),
                                        stop=(kp == 1),
                                        perf_mode=PM.DoubleRow,
                                    )
                        if split_h0:
                            nc.scalar.activation(
                                E[it][:, 1024:2048], Sh[:, 1024:2048], AF.Exp,
                                bias=zb[:, 0:1], scale=EXP_SCALE,
                                accum_out=rsE2[it][:, 1:2])
                        else:
                            nc.scalar.activation(
                                E[it][:, 0:2048], Sh[:], AF.Exp,
                                bias=zb[:, 0:1], scale=EXP_SCALE,
                                accum_out=rsE2[it][:, 1:2])
                        return 3
                    for kp in range(2):
                        for q in (0, 1, 2, 3):
                            c0 = base + q * 512
                            nc.tensor.matmul(
                                Sh[:, q * 512:(q + 1) * 512],
                                lhsT[kp][it],
                                rhs(kp, c0, c0 + 512),
                                start=(kp == 0),
                                stop=(kp == 1),
                                perf_mode=PM.DoubleRow,
                            )
                    if not split_exp:
                        nc.scalar.activation(
                            E[it][:, base:base + 2048], Sh[:], AF.Exp,
                            bias=zb[:, 0:1], scale=EXP_SCALE, accum_out=rsE2[it][:, 2:3])
                        return 3
                    # last tile: split the final exp so the end-of-kernel
                    # dependency chain is one 1024-wide exp shorter.
                    nc.scalar.activation(
                        E[it][:, base:base + 1024], Sh[:, 0:1024], AF.Exp,
                        bias=zb[:, 0:1], scale=EXP_SCALE, accum_out=rsE2[it][:, 2:3])
                    nc.scalar.activation(
                        E[it][:, base + 1024:base + 2048], Sh[:, 1024:2048],
                        AF.Exp, bias=zb[:, 0:1], scale=EXP_SCALE, accum_out=rsE2[it][:, 3:4])
                    return 4

                # ---- pass 1: local half 0 (contains the whole band) ----
                # band = local cols [448 + it*128, +W); masked products run
                # as soon as this tile's half-0 exp lands.
                band_st = []

                def band_mul(it):
                    # negEM = -(E * mask); its row sum lands in rsE2 col 4 so
                    # a single row reduce of rsE2 yields neg directly.
                    Eb = E[it][:, 448 + it * 128: 448 + it * 128 + W]
                    EM1 = bandp.tile([128, W], dt.bfloat16, tag=f"em1_{it}")
                    nc.vector.scalar_tensor_tensor(
                        EM1[:], Eb, -1.0, M1s[:, it * W:(it + 1) * W],
                        ALU.mult, ALU.mult, accum_out=rsE2[it][:, 4:5],
                    )
                    band_st.append(EM1)

                # ---- pass 2 defs (half 1; neg + band ln trail) ----
                negs = {}

                def neg_calc(it, ncols):
                    neg_t = accp.tile([128, 1], dt.float32, tag=f"neg_{it}")
                    nc.vector.tensor_reduce(
                        neg_t[:], rsE2[it][:], AX.X, ALU.add)
                    nc.vector.tensor_copy(out_sb[:, IT + it:IT + it + 1], neg_t[:])
                    negs[it] = neg_t

                def band_ln(it, last=False):
                    if last:
                        Lb = bandp.tile([128, W], dt.bfloat16, tag=f"lb_{it}")
                        nc.scalar.activation(
                            Lb[:], band_st[it][:], AF.Ln, scale=-1.0,
                            bias=negs[it][:, 0:1],
                            accum_out=out_sb[:, it:it + 1],
                        )
                    else:
                        Lb = bandp.tile([128, W], dt.float32, tag=f"lb_{it}")
                        nc.scalar.activation(
                            Lb[:], band_st[it][:], AF.Ln, scale=-1.0,
                            bias=negs[it][:, 0:1],
                        )
                        nc.vector.tensor_reduce(
                            out_sb[:, it:it + 1], Lb[:], AX.X, ALU.add)

                for it in range(IT):
                    gemm_exp(it, 0)
                    band_mul(it)
                for it in range(IT):
                    ncols = gemm_exp(it, 1, split_exp=(it == IT - 1))
                    neg_calc(it, ncols)
                    if it >= 1:
                        band_ln(it - 1)
                band_ln(IT - 1, last=True)
                nc.sync.dma_start(out2[:], out_sb[:])

        nc.finalize()
    finally:
        bacc.get_activation_tables = orig_get
    return nc


def _get_nc():
    if "nc" not in _CACHE:
        _CACHE["nc"] = _build_nc()
    return _CACHE["nc"]


def _host_prep(features, targets):
    f8t = ml_dtypes.float8_e4m3
    f = np.asarray(features, np.float32)
    t = np.asarray(targets).astype(np.int64)

    perm = np.argsort(t, kind="stable")
    fs, ts = f[perm], t[perm]
    rnorm = 1.0 / np.sqrt((fs.astype(np.float64) ** 2).sum(1))
    fn = (fs * rnorm[:, None].astype(np.float32)).astype(np.float32)
    fn8 = (fn * S8).astype(f8t)                     # [N, D] fp8 values
    fnT8 = np.ascontiguousarray(fn8.T)              # [D, N]

    in_maps = []
    for c in range(NCORES):
        roll = np.roll(fnT8, 512 - c * 512, axis=1)     # local col l = global (c*512-512+l) % N
        a = roll.reshape(2, 2, 128, N)                  # [kp, i, p, l]
        im = {}
        for name, c0, c1 in PIECES:
            im[name] = np.ascontiguousarray(
                a[:, :, :, c0:c1].transpose(2, 0, 1, 3))  # [p, kp, i, cols]
        # band masks, local band cols of row tile it: global (R0 - PAD + j) % N
        it_i = np.arange(IT)
        R0 = c * 512 + it_i * 128
        rows = R0[:, None] + np.arange(128)[None, :]            # [IT, p]
        g = (R0[:, None] - PAD + np.arange(W)[None, :]) % N     # [IT, j]
        m1 = (ts[rows][:, :, None] == ts[g][:, None, :])        # [IT, p, j]
        im["m1"] = np.ascontiguousarray(
            m1.transpose(1, 0, 2).reshape(128, IT * W).astype(f8t))
        in_maps.append(im)
    return (fn8, ts), in_maps


def _band_covered(ts):
    """Every same-class pair must fall inside the band (guaranteed for any
    remotely Poisson-like class distribution; checked for safety)."""
    cls, counts = np.unique(ts, return_counts=True)
    starts = np.zeros(len(cls) + 1, np.int64)
    starts[1:] = np.cumsum(counts)
    idx = np.searchsorted(cls, ts)
    row_lo, row_hi = starts[idx], starts[idx] + counts[idx]
    R0 = (np.arange(N) // 128) * 128
    return bool(((row_lo >= R0 - PAD) & (row_hi <= R0 - PAD + W)).all())


def _host_post(fn8, ts, lnsum_rows, neg_rows):
    cls, counts = np.unique(ts, return_counts=True)
    idx = np.searchsorted(cls, ts)
    p = counts[idx].astype(np.float64)
    Eii = np.exp((fn8.astype(np.float64) ** 2).sum(1) / (TAU * S8 * S8))
    A = (lnsum_rows - (W - p) * np.log(neg_rows)
         - np.log(Eii + neg_rows))
    fnq = fn8.astype(np.float64) / S8
    g = np.zeros((len(cls), D), np.float64)
    np.add.at(g, idx, fnq)
    B = ((fnq * g[idx]).sum(1) - (fnq ** 2).sum(1)) / TAU
    loss = ((A - B) / p).sum() / p.sum()
    return np.float32(loss)


def _rows_from_out(per_core_outs):
    lnsum = np.empty(N, np.float64)
    neg = np.empty(N, np.float64)
    for c, out in enumerate(per_core_outs):
        arr = np.asarray(out["out2"], np.float64)      # [128, 2*IT]
        lnsum[c * ROWS:(c + 1) * ROWS] = arr[:, :IT].T.reshape(ROWS)
        neg[c * ROWS:(c + 1) * ROWS] = arr[:, IT:].T.reshape(ROWS)
    return lnsum, neg


def _run(in_maps, trace=False):
    from concourse.bass_utils import run_bass_kernel_spmd
    nc = _get_nc()
    return run_bass_kernel_spmd(
        nc, in_maps, core_ids=list(range(NCORES)), trace=trace,
    )


def _numpy_fallback(features, targets):
    f = np.asarray(features, np.float64)
    t = np.asarray(targets).astype(np.int64)
    sim = f @ f.T
    nrm = np.sqrt((f ** 2).sum(1))
    nm = np.maximum(nrm[:, None] * nrm[None, :], 1e-8)
    E = np.exp(sim / nm / TAU)
    mask = (t[None, :] == t[:, None])
    np.fill_diagonal(E, 0.0)
    negv = (E * ~mask).sum(1)
    p = mask.sum(1).astype(np.float64)
    with np.errstate(divide="ignore"):
        lm = np.where(mask & (E > 0), np.log(E / (E + negv[:, None])), 0.0)
    return np.float32(-(lm / p[:, None]).sum() / p.sum())


def kernel(features, targets):
    (fn8, ts), in_maps = _host_prep(features, targets)
    if not _band_covered(ts):
        return _numpy_fallback(features, targets)
    res = _run(in_maps, trace=False)
    lnsum_rows, neg_rows = _rows_from_out(res.results)
    return _host_post(fn8, ts, lnsum_rows, neg_rows)
